# revision 1
# baseline (speedup 1.0000x reference)
"""Trainium2 Bass/Tile kernel for a BitNet-style fused observation block.

Computation (per reference):
  combined = concat([z_byte, z_addr, z_evt, z_map, z_sum], -1)        # [B, 2048]
  g = TL2(gelu(TL1(combined)))        (TL = ternary-quantized linear)
  t = TL2'(gelu(TL1'(combined)))
  fused = sigmoid(1.2*g) * t
  fused = LN1(fused)
  mlp   = gelu(fused @ mlp_w1.T + b1) @ mlp_w2.T + b2
  out   = LN2(fused + mlp)                                            # [B, 2048]

Strategy: data-parallel over the batch dim (1024 rows per core, 8 cores).
All activations live in transposed [feature, batch] layout on-chip so the
feature dim is the matmul contraction (partition) dim throughout.  Ternary
weights are quantized host-side to {-1, 0, +1} (exactly representable in
bf16); the absmean scale s is folded into the following ScalarEngine
activation as an fp32 immediate, so the matmuls themselves are exact in the
weights.  LayerNorm reductions over features (= partitions) are done on the
TensorEngine with an all-ones [128,128] stationary operand, which yields the
column sums already broadcast across all 128 partitions.

Each core processes its 1024 batch columns in two sequential half-batches of
512 so all live activations fit in SBUF (weights are streamed twice).
"""

import os

import numpy as np
import ml_dtypes

BF16 = ml_dtypes.bfloat16

# Problem dims (hardcoded per the harness contract).
B_TOTAL = 8192
N_CORES = 8
B_CORE = B_TOTAL // N_CORES  # 1024
D = 2048                     # IN == F == 2048
H = 4096                     # MLP hidden
EPS = 1e-5
Z_ORDER = ("z_byte", "z_addr", "z_evt", "z_map", "z_sum")


# ---------------------------------------------------------------------------
# Host-side packing helpers
# ---------------------------------------------------------------------------

def _absmean_scale(w):
    """BitNet absmean scale, matching jnp.mean(jnp.abs(w)) in f32."""
    try:
        import jax
        import jax.numpy as jnp

        cpu = jax.devices("cpu")[0]
        with jax.default_device(cpu):
            s = jnp.mean(jnp.abs(jnp.asarray(w, dtype=jnp.float32)))
            return float(s)
    except Exception:
        return float(np.mean(np.abs(w), dtype=np.float32))


def _ternary(w):
    """Return ({-1,0,1} float32 matrix, scale) for BitNet quantization."""
    w = np.asarray(w, dtype=np.float32)
    s = _absmean_scale(w)
    t = np.round(np.clip(w / np.float32(s + 1e-5), -1.0, 1.0)).astype(np.float32)
    return t, s


def _pack_lhsT(mat, dtype=BF16):
    """Pack [F_out, K] weight into lhsT DRAM layout [ft, ki, ko, f] where
    element = mat[ft*128 + f, ko*128 + ki].  A [128(ki), KT(ko), 128(f)] DMA
    per f-tile then reads per-partition contiguous lines."""
    f_out, k = mat.shape
    arr = mat.reshape(f_out // 128, 128, k // 128, 128)  # [ft, f, ko, ki]
    return np.ascontiguousarray(arr.transpose(0, 3, 2, 1)).astype(dtype)


def _pack_vec(v):
    """Pack per-feature vector [F] into [128, F//128] (partition, f-tile)."""
    v = np.asarray(v, dtype=np.float32)
    return np.ascontiguousarray(v.reshape(-1, 128).T.astype(np.float32))


# ---------------------------------------------------------------------------
# Device program
# ---------------------------------------------------------------------------

def _build(nc, scales, d=D, h=H, b_core=B_CORE, bb=512, gelu_name="Gelu",
           reps=1, tune=None):
    """Emit the per-core Tile program. scales = (sg1, st1, sg2, st2).

    reps > 1 wraps the whole body in a hardware For_i loop (timing builds
    only), so one NEFF execution runs the computation `reps` times."""
    import contextlib
    from contextlib import ExitStack

    import concourse.mybir as mybir
    import concourse.tile as tile

    f32 = mybir.dt.float32
    bf16 = mybir.dt.bfloat16
    AF = mybir.ActivationFunctionType
    OP = mybir.AluOpType
    AF_GELU = getattr(AF, gelu_name)

    tune = dict(tune or {})
    mm_bufs = tune.get("mm_bufs", 4)
    w_bufs = tune.get("w_bufs", 4)
    x16_bufs = tune.get("x16_bufs", 4)
    o_bufs = tune.get("o_bufs", 3)
    stps_bufs = tune.get("stps_bufs", 1)
    bare = tune.get("bare", 0)  # timing probe: skip LN stats+normalize math
    sig_bufs = tune.get("sig_bufs", 3)
    stat_bufs = tune.get("stat_bufs", 2)
    x_slot_bufs = tune.get("x_slot_bufs", 1)
    y_slot_bufs = tune.get("y_slot_bufs", 1)

    sg1, st1, sg2, st2 = scales
    kt = d // 128          # feature tiles of model dim
    ht = h // 128          # feature tiles of mlp hidden
    n_half = b_core // bb  # sequential half-batches

    # --- DRAM I/O -----------------------------------------------------------
    fp8 = mybir.dt.float8e4
    x_d = nc.dram_tensor("x", [kt, 128, b_core], bf16, kind="ExternalInput")
    wg1_d = nc.dram_tensor("wg1", [kt, 128, kt, 128], fp8, kind="ExternalInput")
    wt1_d = nc.dram_tensor("wt1", [kt, 128, kt, 128], fp8, kind="ExternalInput")
    wg2_d = nc.dram_tensor("wg2", [kt, 128, kt, 128], fp8, kind="ExternalInput")
    wt2_d = nc.dram_tensor("wt2", [kt, 128, kt, 128], fp8, kind="ExternalInput")
    wm1_d = nc.dram_tensor("wm1", [ht, 128, kt, 128], bf16, kind="ExternalInput")
    wm2_d = nc.dram_tensor("wm2", [kt, 128, ht, 128], bf16, kind="ExternalInput")
    bg1_d = nc.dram_tensor("bg1", [128, kt], f32, kind="ExternalInput")
    bt1_d = nc.dram_tensor("bt1", [128, kt], f32, kind="ExternalInput")
    bsig_d = nc.dram_tensor("bsig", [128, kt], f32, kind="ExternalInput")  # 1.2*gate_b2
    bt2_d = nc.dram_tensor("bt2", [128, kt], f32, kind="ExternalInput")
    bm1_d = nc.dram_tensor("bm1", [128, ht], f32, kind="ExternalInput")
    bm2_d = nc.dram_tensor("bm2", [128, kt], f32, kind="ExternalInput")
    gln1_d = nc.dram_tensor("gln1", [128, kt], f32, kind="ExternalInput")
    bln1_d = nc.dram_tensor("bln1", [128, kt], f32, kind="ExternalInput")
    gln2_d = nc.dram_tensor("gln2", [128, kt], f32, kind="ExternalInput")
    bln2_d = nc.dram_tensor("bln2", [128, kt], f32, kind="ExternalInput")
    out_d = nc.dram_tensor("outT", [kt, 128, b_core], f32, kind="ExternalOutput")

    x_ap = x_d.ap().rearrange("ko ki b -> ki ko b")
    w_aps = {k: v.ap() for k, v in
             dict(wg1=wg1_d, wt1=wt1_d, wg2=wg2_d, wt2=wt2_d,
                  wm1=wm1_d, wm2=wm2_d).items()}
    out_ap = out_d.ap()

    with tile.TileContext(nc) as tc, ExitStack() as ctx:
        consts = ctx.enter_context(tc.tile_pool(name="consts", bufs=1))
        big = ctx.enter_context(tc.tile_pool(name="big", bufs=1))
        wpool = ctx.enter_context(tc.tile_pool(name="wpool", bufs=w_bufs))
        spool = ctx.enter_context(tc.tile_pool(name="spool", bufs=2))
        opool = ctx.enter_context(tc.tile_pool(name="opool", bufs=o_bufs))
        mm_ps = ctx.enter_context(tc.tile_pool(name="mm_ps", bufs=mm_bufs, space="PSUM"))
        st_ps = ctx.enter_context(tc.tile_pool(name="st_ps", bufs=stps_bufs, space="PSUM"))

        # Constants
        ones16 = consts.tile([128, 128], bf16, name="ones16")
        nc.vector.memset(ones16, 1.0)
        eps_t = consts.tile([128, 1], f32, name="eps_t")
        nc.vector.memset(eps_t, EPS)

        def load_vec(dram, n, name):
            t = consts.tile([128, n], f32, name=name)
            nc.sync.dma_start(t, dram.ap())
            return t

        bg1 = load_vec(bg1_d, kt, "bg1")
        bt1 = load_vec(bt1_d, kt, "bt1")
        bsig = load_vec(bsig_d, kt, "bsig")
        bt2 = load_vec(bt2_d, kt, "bt2")
        bm1 = load_vec(bm1_d, ht, "bm1")
        bm2 = load_vec(bm2_d, kt, "bm2")
        gln1 = load_vec(gln1_d, kt, "gln1")
        bln1 = load_vec(bln1_d, kt, "bln1")
        gln2 = load_vec(gln2_d, kt, "gln2")
        bln2 = load_vec(bln2_d, kt, "bln2")

        def mm_chain(psum, w_tile, rhs_big, n_k):
            for ko in range(n_k):
                nc.tensor.matmul(
                    psum, w_tile[:, ko, :], rhs_big[:, ko, :],
                    start=(ko == 0), stop=(ko == n_k - 1),
                )

        def layernorm(src, gamma, beta, writer, tag):
            """LayerNorm over the feature (partition x f-tile) axis of
            src [128, kt, bb] f32.  Normalized (x-m)*rs*gamma is written back
            into src in place; `writer(j, ap)` then consumes each tile and
            must apply the +beta pass."""
            if bare:
                for j in range(kt):
                    writer(j, src[:, j, :])
                return
            s1 = st_ps.tile([128, bb], f32, tag="s1", name=f"s1_{tag}")
            s2 = st_ps.tile([128, bb], f32, tag="s2", name=f"s2_{tag}")
            for j in range(kt):
                x16 = spool.tile([128, bb], bf16, tag="x16", bufs=x16_bufs, name=f"x16_{tag}_{j}")
                nc.gpsimd.tensor_copy(x16, src[:, j, :])
                xsq = spool.tile([128, bb], bf16, tag="xsq", bufs=x16_bufs, name=f"xsq_{tag}_{j}")
                nc.scalar.activation(xsq, src[:, j, :], AF.Square)
                nc.tensor.matmul(s1, ones16, x16, start=(j == 0), stop=(j == kt - 1))
                nc.tensor.matmul(s2, ones16, xsq, start=(j == 0), stop=(j == kt - 1))
            m = spool.tile([128, bb], f32, tag="m", bufs=stat_bufs, name=f"m_{tag}")
            nc.vector.tensor_scalar_mul(m, s1, 1.0 / d)
            msq = spool.tile([128, bb], f32, tag="msq", bufs=stat_bufs, name=f"msq_{tag}")
            nc.vector.tensor_mul(msq, m, m)
            var = spool.tile([128, bb], f32, tag="var", bufs=stat_bufs, name=f"var_{tag}")
            nc.vector.scalar_tensor_tensor(var, s2, 1.0 / d, msq, OP.mult, OP.subtract)
            sd = spool.tile([128, bb], f32, tag="sd", bufs=stat_bufs, name=f"sd_{tag}")
            nc.scalar.activation(sd, var, AF.Sqrt, bias=eps_t[:, 0:1], scale=1.0)
            rs = spool.tile([128, bb], f32, tag="rs", bufs=stat_bufs, name=f"rs_{tag}")
            nc.vector.reciprocal(rs, sd)
            for j in range(kt):
                nc.vector.tensor_sub(src[:, j, :], src[:, j, :], m)
                nc.vector.scalar_tensor_tensor(
                    src[:, j, :], src[:, j, :], gamma[:, j:j + 1], rs, OP.mult, OP.mult)
                writer(j, src[:, j, :])

        if reps > 1:
            loop_ctx = tc.For_i(0, reps, 1,
                                hint_engines=tuple(nc.engines.keys()))
        else:
            loop_ctx = contextlib.nullcontext()
        ctx.enter_context(loop_ctx)

        for half in range(n_half):
            b0 = half * bb
            hb = f"h{half}"

            # --- load activations (transposed, bf16) -------------------
            xsb = big.tile([128, kt, bb], bf16, tag="X", bufs=x_slot_bufs, name=f"xsb_{hb}")
            nc.sync.dma_start(xsb, x_ap[:, :, b0:b0 + bb])

            # --- phase A: y1{g,t} = gelu(s1 * (W1 @ x) + b1) -----------
            y1g = big.tile([128, kt, bb], bf16, tag="Yg", bufs=y_slot_bufs, name=f"y1g_{hb}")
            y1t = big.tile([128, kt, bb], bf16, tag="Yt", bufs=y_slot_bufs, name=f"y1t_{hb}")
            for ft in range(kt):
                for wkey, y, bias, scale in (
                    ("wg1", y1g, bg1, sg1), ("wt1", y1t, bt1, st1)):
                    wt = wpool.tile([128, kt, 128], fp8, tag="W",
                                    name=f"w_{wkey}_{hb}_{ft}")
                    nc.sync.dma_start(wt, w_aps[wkey][ft])
                    ps = mm_ps.tile([128, bb], f32, tag="mm", name=f"psA_{wkey}_{hb}_{ft}")
                    mm_chain(ps, wt, xsb, kt)
                    nc.scalar.activation(y[:, ft, :], ps, AF_GELU,
                                         bias=bias[:, ft:ft + 1], scale=scale)

            # --- phase B: fused = sigmoid(1.2*g) * t -------------------
            fused = big.tile([128, kt, bb], f32, tag="F", name=f"fused_{hb}")
            for ft in range(kt):
                wg = wpool.tile([128, kt, 128], fp8, tag="W", name=f"w_wg2_{hb}_{ft}")
                nc.sync.dma_start(wg, w_aps["wg2"][ft])
                psg = mm_ps.tile([128, bb], f32, tag="mm", name=f"psBg_{hb}_{ft}")
                mm_chain(psg, wg, y1g, kt)
                wt2 = wpool.tile([128, kt, 128], fp8, tag="W", name=f"w_wt2_{hb}_{ft}")
                nc.sync.dma_start(wt2, w_aps["wt2"][ft])
                pst = mm_ps.tile([128, bb], f32, tag="mm", name=f"psBt_{hb}_{ft}")
                mm_chain(pst, wt2, y1t, kt)
                sig = spool.tile([128, bb], f32, tag="sig", bufs=sig_bufs, name=f"sig_{hb}_{ft}")
                nc.scalar.activation(sig, psg, AF.Sigmoid,
                                     bias=bsig[:, ft:ft + 1], scale=1.2 * sg2)
                tt = spool.tile([128, bb], f32, tag="tt", bufs=sig_bufs, name=f"tt_{hb}_{ft}")
                nc.scalar.activation(tt, pst, AF.Identity,
                                     bias=bt2[:, ft:ft + 1], scale=st2)
                nc.vector.tensor_mul(fused[:, ft, :], sig, tt)

            # --- LN1 (in place on fused); bf16 copy for the MLP --------
            f16 = big.tile([128, kt, bb], bf16, tag="F16", name=f"f16_{hb}")

            def ln1_writer(j, xn):
                nc.scalar.activation(xn, xn, AF.Identity,
                                     bias=bln1[:, j:j + 1], scale=1.0)
                nc.gpsimd.tensor_copy(f16[:, j, :], xn)

            layernorm(fused, gln1, bln1, ln1_writer, f"ln1{hb}")

            # --- phase D: hmid = gelu(f16 @ mlp_w1.T + b1) -------------
            hsb = big.tile([128, ht, bb], bf16, tag="Hm", name=f"hsb_{hb}")
            for ft in range(ht):
                wt = wpool.tile([128, kt, 128], bf16, tag="W", name=f"w_wm1_{hb}_{ft}")
                nc.sync.dma_start(wt, w_aps["wm1"][ft])
                ps = mm_ps.tile([128, bb], f32, tag="mm", name=f"psD_{hb}_{ft}")
                mm_chain(ps, wt, f16, kt)
                nc.scalar.activation(hsb[:, ft, :], ps, AF_GELU,
                                     bias=bm1[:, ft:ft + 1], scale=1.0)

            # --- phase E: fused += hmid @ mlp_w2.T + b2 ----------------
            for ft in range(kt):
                wt = wpool.tile([128, ht, 128], bf16, tag="W", name=f"w_wm2_{hb}_{ft}")
                nc.sync.dma_start(wt, w_aps["wm2"][ft])
                ps = mm_ps.tile([128, bb], f32, tag="mm", name=f"psE_{hb}_{ft}")
                mm_chain(ps, wt, hsb, ht)
                nc.vector.scalar_tensor_tensor(
                    fused[:, ft, :], ps, bm2[:, ft:ft + 1], fused[:, ft, :],
                    OP.add, OP.add)

            # --- LN2 -> output -----------------------------------------
            def ln2_writer(j, xn):
                ot = opool.tile([128, bb], f32, tag="o", name=f"ot_{hb}_{j}")
                nc.scalar.activation(ot, xn, AF.Identity,
                                     bias=bln2[:, j:j + 1], scale=1.0)
                nc.sync.dma_start(out_ap[j, :, b0:b0 + bb], ot)

            layernorm(fused, gln2, bln2, ln2_writer, f"ln2{hb}")

    return nc


# ---------------------------------------------------------------------------
# Host entry point
# ---------------------------------------------------------------------------

def _prep(inputs, d=D, h=H, b_total=B_TOTAL, n_cores=N_CORES):
    """Host-side marshalling: concat+transpose activations, ternary-quantize
    and pack weights.  Returns (per-core input maps, scales)."""
    zs = [np.asarray(inputs[k], dtype=np.float32) for k in Z_ORDER if k in inputs]
    combined = np.concatenate(zs, axis=1)  # [B, D]
    assert combined.shape == (b_total, d), combined.shape
    xt = np.ascontiguousarray(combined.T.astype(BF16))  # [D, B]

    tg1, sg1 = _ternary(inputs["gate_w1"])
    tt1, st1 = _ternary(inputs["tr_w1"])
    tg2, sg2 = _ternary(inputs["gate_w2"])
    tt2, st2 = _ternary(inputs["tr_w2"])

    import concourse.mybir as _mybir

    fp8 = _mybir.dt.np(_mybir.dt.float8e4)  # ternary {-1,0,1} is exact in fp8
    shared = {
        "wg1": _pack_lhsT(tg1, fp8),
        "wt1": _pack_lhsT(tt1, fp8),
        "wg2": _pack_lhsT(tg2, fp8),
        "wt2": _pack_lhsT(tt2, fp8),
        "wm1": _pack_lhsT(np.asarray(inputs["mlp_w1"], dtype=np.float32)),
        "wm2": _pack_lhsT(np.asarray(inputs["mlp_w2"], dtype=np.float32)),
        "bg1": _pack_vec(inputs["gate_b1"]),
        "bt1": _pack_vec(inputs["tr_b1"]),
        "bsig": _pack_vec(np.asarray(inputs["gate_b2"], np.float32) * np.float32(1.2)),
        "bt2": _pack_vec(inputs["tr_b2"]),
        "bm1": _pack_vec(inputs["mlp_b1"]),
        "bm2": _pack_vec(inputs["mlp_b2"]),
        "gln1": _pack_vec(inputs["ln1_g"]),
        "bln1": _pack_vec(inputs["ln1_b"]),
        "gln2": _pack_vec(inputs["ln2_g"]),
        "bln2": _pack_vec(inputs["ln2_b"]),
    }

    b_core = b_total // n_cores
    kt = d // 128
    in_maps = []
    for c in range(n_cores):
        xc = np.ascontiguousarray(
            xt[:, c * b_core:(c + 1) * b_core].reshape(kt, 128, b_core))
        in_maps.append({"x": xc, **shared})
    return in_maps, (sg1, st1, sg2, st2)


def _assemble(results, d=D, b_total=B_TOTAL, n_cores=N_CORES):
    b_core = b_total // n_cores
    out = np.empty((b_total, d), dtype=np.float32)
    for c, r in enumerate(results):
        # outT [kt, 128, b_core] -> [b_core, d]
        out[c * b_core:(c + 1) * b_core] = (
            r["outT"].transpose(2, 0, 1).reshape(b_core, d))
    return out


def _make_nc(num_devices=N_CORES):
    from concourse import bacc

    return bacc.Bacc("TRN2", target_bir_lowering=False, debug=False,
                     enable_asserts=False, num_devices=num_devices)


def kernel(**inputs):
    os.environ.setdefault("BASS_NEVER_TRACE", "1")
    from concourse.bass_utils import run_bass_kernel_spmd

    in_maps, scales = _prep(inputs)
    nc = _make_nc()
    _build(nc, scales)
    nc.compile()
    res = run_bass_kernel_spmd(nc, in_maps, core_ids=list(range(N_CORES)))
    return _assemble(res.results)



# revision 4
# speedup vs baseline: 2.3066x; 2.3066x over previous
"""Trainium2 Bass/Tile kernel for a BitNet-style fused observation block.

Computation (per reference):
  combined = concat([z_byte, z_addr, z_evt, z_map, z_sum], -1)        # [B, 2048]
  g = TL2(gelu(TL1(combined)))        (TL = ternary-quantized linear)
  t = TL2'(gelu(TL1'(combined)))
  fused = sigmoid(1.2*g) * t
  fused = LN1(fused)
  mlp   = gelu(fused @ mlp_w1.T + b1) @ mlp_w2.T + b2
  out   = LN2(fused + mlp)                                            # [B, 2048]

Strategy (v2): data-parallel over batch (1024 rows/core, 8 cores), activations
transposed [feature, batch] so features are the contraction dim.

Key speedups over v1:
  * Ternary matmuls run in fp8 DoubleRow mode (K=256 per instruction,
    ~105ns/MM vs 213ns bf16 => 2x).  Ternary weights are exact in fp8; the
    activations are sent as a split pair x = x8 + xr8 (both fp8e4m3), so the
    pair of half-chains reproduces bf16-level accuracy while running at fp8
    DoubleRow speed.
  * LN1 is folded into the mlp_w1 matmul: W1 @ LN1(f) = rs*(W1g @ f)
    - m*rs*c1 + c3 with W1g = mlp_w1*diag(ln1_g), c1 = mlp_w1@ln1_g,
    c3 = mlp_w1@ln1_b + mlp_b1 (host-precomputed).  Removes the LN1
    normalize barrier between the gate/transform part and the MLP.
  * sigmoid(1.2g) computed as 0.5*tanh(0.6g)+0.5 (tanh lives in the same
    ACT table set as gelu -> fewer ~2.7us table switches).
  * LN2 normalize+store of half h is emitted after half h+1's first matmul
    phase, so it overlaps the tensor engine instead of serializing.

LayerNorm reductions over features (= partitions) use the TensorEngine with
an all-ones [128,128] stationary operand (column sums broadcast across
partitions).  Each core processes its 1024 batch columns in two sequential
half-batches of 512 (weights streamed twice).
"""

import os

import numpy as np
import ml_dtypes

BF16 = ml_dtypes.bfloat16

# Problem dims (hardcoded per the harness contract).
B_TOTAL = 8192
N_CORES = 8
B_CORE = B_TOTAL // N_CORES  # 1024
D = 2048                     # IN == F == 2048
H = 4096                     # MLP hidden
EPS = 1e-5
Z_ORDER = ("z_byte", "z_addr", "z_evt", "z_map", "z_sum")


# ---------------------------------------------------------------------------
# Host-side packing helpers
# ---------------------------------------------------------------------------

def _absmean_scale(w):
    """BitNet absmean scale, matching jnp.mean(jnp.abs(w)) in f32."""
    try:
        import jax
        import jax.numpy as jnp

        cpu = jax.devices("cpu")[0]
        with jax.default_device(cpu):
            s = jnp.mean(jnp.abs(jnp.asarray(w, dtype=jnp.float32)))
            return float(s)
    except Exception:
        return float(np.mean(np.abs(w), dtype=np.float32))


def _ternary(w):
    """Return ({-1,0,1} float32 matrix, scale) for BitNet quantization."""
    w = np.asarray(w, dtype=np.float32)
    s = _absmean_scale(w)
    t = np.round(np.clip(w / np.float32(s + 1e-5), -1.0, 1.0)).astype(np.float32)
    return t, s


def _pack_lhsT(mat, dtype=BF16):
    """Pack [F_out, K] weight into lhsT DRAM layout [ft, ki, ko, f] where
    element = mat[ft*128 + f, ko*128 + ki]."""
    f_out, k = mat.shape
    arr = mat.reshape(f_out // 128, 128, k // 128, 128)  # [ft, f, ko, ki]
    return np.ascontiguousarray(arr.transpose(0, 3, 2, 1)).astype(dtype)


def _pack_vec(v):
    """Pack per-feature vector [F] into [128, F//128] (partition, f-tile)."""
    v = np.asarray(v, dtype=np.float32)
    return np.ascontiguousarray(v.reshape(-1, 128).T.astype(np.float32))


# ---------------------------------------------------------------------------
# Device program
# ---------------------------------------------------------------------------

def _build(nc, scales, d=D, h=H, b_core=B_CORE, bb=512, reps=1, tune=None):
    """Emit the per-core Tile program. scales = (sg1, st1, sg2, st2)."""
    import contextlib
    from contextlib import ExitStack

    import concourse.mybir as mybir
    import concourse.tile as tile

    f32 = mybir.dt.float32
    bf16 = mybir.dt.bfloat16
    fp8 = mybir.dt.float8e4
    AF = mybir.ActivationFunctionType
    OP = mybir.AluOpType
    DR = mybir.MatmulPerfMode.DoubleRow

    tune = dict(tune or {})
    w_bufs = tune.get("w_bufs", 3)
    mm_bufs = tune.get("mm_bufs", 4)
    o_bufs = tune.get("o_bufs", 3)
    sp_bufs = tune.get("sp_bufs", 2)
    interleave = tune.get("interleave", 1)

    sg1, st1, sg2, st2 = scales
    kt = d // 128          # 16 feature tiles of model dim
    ht = h // 128          # 32 feature tiles of mlp hidden
    kp = kt // 2           # 8 DoubleRow k-pairs
    n_half = b_core // bb  # sequential half-batches

    # --- DRAM I/O -----------------------------------------------------------
    x8_d = nc.dram_tensor("x8", [kt, 128, b_core], fp8, kind="ExternalInput")
    xr8_d = nc.dram_tensor("xr8", [kt, 128, b_core], fp8, kind="ExternalInput")
    wg1_d = nc.dram_tensor("wg1", [kt, 128, kt, 128], fp8, kind="ExternalInput")
    wt1_d = nc.dram_tensor("wt1", [kt, 128, kt, 128], fp8, kind="ExternalInput")
    wg2_d = nc.dram_tensor("wg2", [kt, 128, kt, 128], fp8, kind="ExternalInput")
    wt2_d = nc.dram_tensor("wt2", [kt, 128, kt, 128], fp8, kind="ExternalInput")
    wm1_d = nc.dram_tensor("wm1", [ht, 128, kt, 128], bf16, kind="ExternalInput")
    wm2_d = nc.dram_tensor("wm2", [kt, 128, ht, 128], bf16, kind="ExternalInput")
    bg1_d = nc.dram_tensor("bg1", [128, kt], f32, kind="ExternalInput")
    bt1_d = nc.dram_tensor("bt1", [128, kt], f32, kind="ExternalInput")
    btanh_d = nc.dram_tensor("btanh", [128, kt], f32, kind="ExternalInput")
    bthalf_d = nc.dram_tensor("bthalf", [128, kt], f32, kind="ExternalInput")
    negc1_d = nc.dram_tensor("negc1", [128, ht], f32, kind="ExternalInput")
    c3_d = nc.dram_tensor("c3", [128, ht], f32, kind="ExternalInput")
    bm2b1_d = nc.dram_tensor("bm2b1", [128, kt], f32, kind="ExternalInput")
    g1s_d = nc.dram_tensor("g1s", [128, kt], f32, kind="ExternalInput")
    gln2_d = nc.dram_tensor("gln2", [128, kt], f32, kind="ExternalInput")
    bln2_d = nc.dram_tensor("bln2", [128, kt], f32, kind="ExternalInput")
    out_d = nc.dram_tensor("outT", [kt, 128, b_core], f32, kind="ExternalOutput")

    x8_ap = x8_d.ap().rearrange("ko ki b -> ki ko b")
    xr8_ap = xr8_d.ap().rearrange("ko ki b -> ki ko b")
    w_aps = {k: v.ap() for k, v in
             dict(wg1=wg1_d, wt1=wt1_d, wg2=wg2_d, wt2=wt2_d,
                  wm1=wm1_d, wm2=wm2_d).items()}
    out_ap = out_d.ap()

    with tile.TileContext(nc) as tc, ExitStack() as ctx:
        consts = ctx.enter_context(tc.tile_pool(name="consts", bufs=1))
        big = ctx.enter_context(tc.tile_pool(name="big", bufs=1))
        wpool = ctx.enter_context(tc.tile_pool(name="wpool", bufs=w_bufs))
        spool = ctx.enter_context(tc.tile_pool(name="spool", bufs=2))
        opool = ctx.enter_context(tc.tile_pool(name="opool", bufs=o_bufs))
        mm_ps = ctx.enter_context(tc.tile_pool(name="mm_ps", bufs=mm_bufs, space="PSUM"))
        st_ps = ctx.enter_context(tc.tile_pool(name="st_ps", bufs=1, space="PSUM"))

        # Constants
        ones16 = consts.tile([128, 128], bf16, name="ones16")
        nc.vector.memset(ones16, 1.0)
        eps_t = consts.tile([128, 1], f32, name="eps_t")
        nc.vector.memset(eps_t, EPS)

        def load_vec(dram, n, name):
            t = consts.tile([128, n], f32, name=name)
            nc.sync.dma_start(t, dram.ap())
            return t

        bg1 = load_vec(bg1_d, kt, "bg1")
        bt1 = load_vec(bt1_d, kt, "bt1")
        btanh = load_vec(btanh_d, kt, "btanh")
        bthalf = load_vec(bthalf_d, kt, "bthalf")
        negc1 = load_vec(negc1_d, ht, "negc1")
        c3 = load_vec(c3_d, ht, "c3")
        bm2b1 = load_vec(bm2b1_d, kt, "bm2b1")
        g1s = load_vec(g1s_d, kt, "g1s")
        gln2 = load_vec(gln2_d, kt, "gln2")
        bln2 = load_vec(bln2_d, kt, "bln2")

        def dr_chain(psum, w_tile, rhs8, rhsr8):
            """16 DoubleRow matmuls accumulating W @ (x8 + xr8), K=2048."""
            for j, rhs in ((0, rhs8), (1, rhsr8)):
                for ko in range(kp):
                    nc.tensor.matmul(
                        psum, w_tile[:, 2 * ko:2 * ko + 2, :],
                        rhs[:, 2 * ko:2 * ko + 2, :],
                        start=(j == 0 and ko == 0),
                        stop=(j == 1 and ko == kp - 1),
                        perf_mode=DR,
                    )

        def mm_chain(psum, w_tile, rhs_big, n_k):
            for ko in range(n_k):
                nc.tensor.matmul(
                    psum, w_tile[:, ko, :], rhs_big[:, ko, :],
                    start=(ko == 0), stop=(ko == n_k - 1),
                )

        # Per-half state (tiles are allocated lazily per half; tags+bufs=1
        # make half h+1 reuse half h's buffers with auto dependencies).
        state = {}

        def emit_A(half):
            b0 = half * bb
            hb = f"h{half}"
            xs8 = big.tile([128, kt, bb], fp8, tag="X8", name=f"xs8_{hb}")
            xsr8 = big.tile([128, kt, bb], fp8, tag="XR8", name=f"xsr8_{hb}")
            nc.sync.dma_start(xs8, x8_ap[:, :, b0:b0 + bb])
            nc.sync.dma_start(xsr8, xr8_ap[:, :, b0:b0 + bb])
            y8g = big.tile([128, kt, bb], fp8, tag="Y8g", name=f"y8g_{hb}")
            yr8g = big.tile([128, kt, bb], fp8, tag="YR8g", name=f"yr8g_{hb}")
            y8t = big.tile([128, kt, bb], fp8, tag="Y8t", name=f"y8t_{hb}")
            yr8t = big.tile([128, kt, bb], fp8, tag="YR8t", name=f"yr8t_{hb}")
            for ft in range(kt):
                for wkey, y8, yr8, bias, scale in (
                    ("wg1", y8g, yr8g, bg1, sg1), ("wt1", y8t, yr8t, bt1, st1)):
                    wt = wpool.tile([128, kt, 128], fp8, tag="W8",
                                    name=f"w_{wkey}_{hb}_{ft}")
                    nc.sync.dma_start(wt, w_aps[wkey][ft])
                    ps = mm_ps.tile([128, bb], f32, tag="mm",
                                    name=f"psA_{wkey}_{hb}_{ft}")
                    dr_chain(ps, wt, xs8, xsr8)
                    ytmp = spool.tile([128, bb], bf16, tag="ytmp", bufs=sp_bufs,
                                      name=f"ytmp_{wkey}_{hb}_{ft}")
                    nc.scalar.activation(ytmp, ps, AF.Gelu,
                                         bias=bias[:, ft:ft + 1], scale=scale)
                    nc.gpsimd.tensor_copy(y8[:, ft, :], ytmp)
                    nc.vector.tensor_sub(yr8[:, ft, :], ytmp, y8[:, ft, :])
            state[half] = dict(y8g=y8g, yr8g=yr8g, y8t=y8t, yr8t=yr8t)

        def stats(src_big, tag, hb):
            """Column mean + rsqrt(var+eps) of [128, kt, bb] bf16 via TensorE.
            Returns (m, rs) f32 [128, bb] tiles (broadcast across partitions)."""
            s1 = st_ps.tile([128, bb], f32, tag=f"s1{tag}", name=f"s1_{tag}_{hb}")
            s2 = st_ps.tile([128, bb], f32, tag=f"s2{tag}", name=f"s2_{tag}_{hb}")
            for j in range(kt):
                xsq = spool.tile([128, bb], bf16, tag="xsq", bufs=sp_bufs,
                                 name=f"xsq_{tag}_{hb}_{j}")
                nc.scalar.activation(xsq, src_big[:, j, :], AF.Square)
                nc.tensor.matmul(s1, ones16, src_big[:, j, :],
                                 start=(j == 0), stop=(j == kt - 1))
                nc.tensor.matmul(s2, ones16, xsq,
                                 start=(j == 0), stop=(j == kt - 1))
            m = spool.tile([128, bb], f32, tag=f"m{tag}", bufs=1,
                           name=f"m_{tag}_{hb}")
            nc.vector.tensor_scalar_mul(m, s1, 1.0 / d)
            msq = spool.tile([128, bb], f32, tag="msq", bufs=2,
                             name=f"msq_{tag}_{hb}")
            nc.vector.tensor_mul(msq, m, m)
            var = spool.tile([128, bb], f32, tag="var", bufs=2,
                             name=f"var_{tag}_{hb}")
            nc.vector.scalar_tensor_tensor(var, s2, 1.0 / d, msq,
                                           OP.mult, OP.subtract)
            sd = spool.tile([128, bb], f32, tag="sd", bufs=2,
                            name=f"sd_{tag}_{hb}")
            nc.scalar.activation(sd, var, AF.Sqrt, bias=eps_t[:, 0:1], scale=1.0)
            rs = spool.tile([128, bb], f32, tag=f"rs{tag}", bufs=1,
                            name=f"rs_{tag}_{hb}")
            nc.vector.reciprocal(rs, sd)
            return m, rs

        def emit_B(half):
            hb = f"h{half}"
            st = state[half]
            fr16 = big.tile([128, kt, bb], bf16, tag="F16", name=f"fr16_{hb}")
            for ft in range(kt):
                wg = wpool.tile([128, kt, 128], fp8, tag="W8",
                                name=f"w_wg2_{hb}_{ft}")
                nc.sync.dma_start(wg, w_aps["wg2"][ft])
                psg = mm_ps.tile([128, bb], f32, tag="mm", name=f"psBg_{hb}_{ft}")
                dr_chain(psg, wg, st["y8g"], st["yr8g"])
                wt2 = wpool.tile([128, kt, 128], fp8, tag="W8",
                                 name=f"w_wt2_{hb}_{ft}")
                nc.sync.dma_start(wt2, w_aps["wt2"][ft])
                pst = mm_ps.tile([128, bb], f32, tag="mm", name=f"psBt_{hb}_{ft}")
                dr_chain(pst, wt2, st["y8t"], st["yr8t"])
                tanhv = spool.tile([128, bb], bf16, tag="tanhv", bufs=sp_bufs,
                                   name=f"tanhv_{hb}_{ft}")
                nc.scalar.activation(tanhv, psg, AF.Tanh,
                                     bias=btanh[:, ft:ft + 1], scale=0.6 * sg2)
                tth = spool.tile([128, bb], bf16, tag="tth", bufs=sp_bufs,
                                 name=f"tth_{hb}_{ft}")
                nc.scalar.activation(tth, pst, AF.Identity,
                                     bias=bthalf[:, ft:ft + 1], scale=0.5 * st2)
                # fused = sigmoid(1.2g)*t = (tanh(0.6g)+1) * (t/2)
                nc.vector.scalar_tensor_tensor(
                    fr16[:, ft, :], tanhv, 1.0, tth, OP.add, OP.mult)
            state[half]["fr16"] = fr16
            m, rs = stats(fr16, "a", hb)
            state[half]["m1"] = m
            state[half]["rs1"] = rs

        def emit_D(half):
            hb = f"h{half}"
            st = state[half]
            fr16, m1, rs1 = st["fr16"], st["m1"], st["rs1"]
            hsb = big.tile([128, ht, bb], bf16, tag="Hm", name=f"hsb_{hb}")
            for ft in range(ht):
                wt = wpool.tile([128, kt, 128], bf16, tag="Wd",
                                name=f"w_wm1_{hb}_{ft}")
                nc.sync.dma_start(wt, w_aps["wm1"][ft])
                ps = mm_ps.tile([128, bb], f32, tag="mm", name=f"psD_{hb}_{ft}")
                mm_chain(ps, wt, fr16, kt)
                # u = rs*(P - m*c1) + c3 ; hmid = gelu(u)
                u1 = spool.tile([128, bb], f32, tag="u1", bufs=sp_bufs,
                                name=f"u1_{hb}_{ft}")
                nc.vector.scalar_tensor_tensor(
                    u1, m1, negc1[:, ft:ft + 1], ps, OP.mult, OP.add)
                u2 = spool.tile([128, bb], bf16, tag="u2", bufs=sp_bufs,
                                name=f"u2_{hb}_{ft}")
                nc.gpsimd.tensor_mul(u2, u1, rs1)
                nc.scalar.activation(hsb[:, ft, :], u2, AF.Gelu,
                                     bias=c3[:, ft:ft + 1], scale=1.0)
            state[half]["hsb"] = hsb

        def emit_E(half):
            hb = f"h{half}"
            st = state[half]
            fr16, m1, rs1, hsb = st["fr16"], st["m1"], st["rs1"], st["hsb"]
            for ft in range(kt):
                wt = wpool.tile([128, ht, 128], bf16, tag="We",
                                name=f"w_wm2_{hb}_{ft}")
                nc.sync.dma_start(wt, w_aps["wm2"][ft])
                ps = mm_ps.tile([128, bb], f32, tag="mm", name=f"psE_{hb}_{ft}")
                mm_chain(ps, wt, hsb, ht)
                # resid = (fr16 - m1) * g1 * rs1  (LN1 output minus bias)
                r1 = spool.tile([128, bb], f32, tag="r1", bufs=sp_bufs,
                                name=f"r1_{hb}_{ft}")
                nc.gpsimd.tensor_sub(r1, fr16[:, ft, :], m1)
                r2 = spool.tile([128, bb], f32, tag="r2", bufs=sp_bufs,
                                name=f"r2_{hb}_{ft}")
                nc.vector.scalar_tensor_tensor(
                    r2, r1, g1s[:, ft:ft + 1], rs1, OP.mult, OP.mult)
                # z = P + (mlp_b2 + ln1_b) + resid, overwrites fr16[ft]
                nc.vector.scalar_tensor_tensor(
                    fr16[:, ft, :], ps, bm2b1[:, ft:ft + 1], r2, OP.add, OP.add)

        def emit_LN2stats(half):
            hb = f"h{half}"
            st = state[half]
            m2, rs2 = stats(st["fr16"], "b", hb)
            state[half]["m2"] = m2
            state[half]["rs2"] = rs2

        def emit_LN2norm(half):
            hb = f"h{half}"
            b0 = half * bb
            st = state[half]
            z, m2, rs2 = st["fr16"], st["m2"], st["rs2"]
            for ft in range(kt):
                o1 = spool.tile([128, bb], f32, tag="o1", bufs=sp_bufs,
                                name=f"o1_{hb}_{ft}")
                nc.gpsimd.tensor_sub(o1, z[:, ft, :], m2)
                o2 = spool.tile([128, bb], f32, tag="o2", bufs=sp_bufs,
                                name=f"o2_{hb}_{ft}")
                nc.vector.scalar_tensor_tensor(
                    o2, o1, gln2[:, ft:ft + 1], rs2, OP.mult, OP.mult)
                ot = opool.tile([128, bb], f32, tag="o", name=f"ot_{hb}_{ft}")
                nc.scalar.activation(ot, o2, AF.Identity,
                                     bias=bln2[:, ft:ft + 1], scale=1.0)
                nc.sync.dma_start(out_ap[ft, :, b0:b0 + bb], ot)

        if reps > 1:
            loop_ctx = tc.For_i(0, reps, 1,
                                hint_engines=tuple(nc.engines.keys()))
        else:
            loop_ctx = contextlib.nullcontext()
        ctx.enter_context(loop_ctx)

        for half in range(n_half):
            emit_A(half)
            if interleave and half > 0:
                emit_LN2norm(half - 1)
            emit_B(half)
            emit_D(half)
            emit_E(half)
            emit_LN2stats(half)
            if not interleave:
                emit_LN2norm(half)
        if interleave:
            emit_LN2norm(n_half - 1)

    return nc


# ---------------------------------------------------------------------------
# Host entry point
# ---------------------------------------------------------------------------

def _prep(inputs, d=D, h=H, b_total=B_TOTAL, n_cores=N_CORES):
    """Host-side marshalling: concat+transpose activations (split into an
    fp8 pair), ternary-quantize + pack weights, fold LN1 into mlp_w1."""
    zs = [np.asarray(inputs[k], dtype=np.float32) for k in Z_ORDER if k in inputs]
    combined = np.concatenate(zs, axis=1)  # [B, D]
    assert combined.shape == (b_total, d), combined.shape

    import concourse.mybir as _mybir

    fp8 = _mybir.dt.np(_mybir.dt.float8e4)

    xt = np.ascontiguousarray(combined.T)          # [D, B] f32
    x8 = xt.astype(fp8)
    xr8 = (xt - x8.astype(np.float32)).astype(fp8)

    tg1, sg1 = _ternary(inputs["gate_w1"])
    tt1, st1 = _ternary(inputs["tr_w1"])
    tg2, sg2 = _ternary(inputs["gate_w2"])
    tt2, st2 = _ternary(inputs["tr_w2"])

    mlp_w1 = np.asarray(inputs["mlp_w1"], dtype=np.float32)
    mlp_w2 = np.asarray(inputs["mlp_w2"], dtype=np.float32)
    g1 = np.asarray(inputs["ln1_g"], dtype=np.float32)
    b1 = np.asarray(inputs["ln1_b"], dtype=np.float32)
    w1g = mlp_w1 * g1[None, :]
    c1 = mlp_w1 @ g1
    c3 = mlp_w1 @ b1 + np.asarray(inputs["mlp_b1"], np.float32)

    shared = {
        "wg1": _pack_lhsT(tg1, fp8),
        "wt1": _pack_lhsT(tt1, fp8),
        "wg2": _pack_lhsT(tg2, fp8),
        "wt2": _pack_lhsT(tt2, fp8),
        "wm1": _pack_lhsT(w1g),
        "wm2": _pack_lhsT(mlp_w2),
        "bg1": _pack_vec(inputs["gate_b1"]),
        "bt1": _pack_vec(inputs["tr_b1"]),
        "btanh": _pack_vec(np.asarray(inputs["gate_b2"], np.float32) * np.float32(0.6)),
        "bthalf": _pack_vec(np.asarray(inputs["tr_b2"], np.float32) * np.float32(0.5)),
        "negc1": _pack_vec(-c1),
        "c3": _pack_vec(c3),
        "bm2b1": _pack_vec(np.asarray(inputs["mlp_b2"], np.float32) + b1),
        "g1s": _pack_vec(g1),
        "gln2": _pack_vec(inputs["ln2_g"]),
        "bln2": _pack_vec(inputs["ln2_b"]),
    }

    b_core = b_total // n_cores
    kt = d // 128
    in_maps = []
    for c in range(n_cores):
        sl = slice(c * b_core, (c + 1) * b_core)
        in_maps.append({
            "x8": np.ascontiguousarray(x8[:, sl].reshape(kt, 128, b_core)),
            "xr8": np.ascontiguousarray(xr8[:, sl].reshape(kt, 128, b_core)),
            **shared,
        })
    return in_maps, (sg1, st1, sg2, st2)


def _assemble(results, d=D, b_total=B_TOTAL, n_cores=N_CORES):
    b_core = b_total // n_cores
    out = np.empty((b_total, d), dtype=np.float32)
    for c, r in enumerate(results):
        out[c * b_core:(c + 1) * b_core] = (
            r["outT"].transpose(2, 0, 1).reshape(b_core, d))
    return out


def _make_nc(num_devices=N_CORES):
    from concourse import bacc

    return bacc.Bacc("TRN2", target_bir_lowering=False, debug=False,
                     enable_asserts=False, num_devices=num_devices)


def kernel(**inputs):
    os.environ.setdefault("BASS_NEVER_TRACE", "1")
    from concourse.bass_utils import run_bass_kernel_spmd

    in_maps, scales = _prep(inputs)
    nc = _make_nc()
    _build(nc, scales)
    nc.compile()
    res = run_bass_kernel_spmd(nc, in_maps, core_ids=list(range(N_CORES)))
    return _assemble(res.results)
